# revision 31
# baseline (speedup 1.0000x reference)
"""Fused attention block (QKV -> masked softmax -> AV+residual -> out-proj -> LN)
for one trn2 chip (8 NeuronCores), sequence-parallel over the 4096 rows.

Self-contained: hardcodes shapes/sharding; builds+compiles the Bass program on
first call and caches it.
"""
import sys

sys.path.insert(0, "/opt/trn_rl_repo")

from contextlib import ExitStack  # noqa: E402

import numpy as np  # noqa: E402

import concourse.bacc as bacc  # noqa: E402
import concourse.bass as bass  # noqa: E402
import concourse.tile as tile  # noqa: E402
from concourse import mybir  # noqa: E402
from concourse.bass_utils import run_bass_kernel_spmd  # noqa: E402

f32 = mybir.dt.float32
f32r = mybir.dt.float32r
u8 = mybir.dt.uint8
bf = mybir.dt.bfloat16
AF = mybir.ActivationFunctionType
OP = mybir.AluOpType
AX = mybir.AxisListType
ts = bass.ts

NCORES = 8
S = 4096          # full sequence
D = 1024          # d_model
SH = S // NCORES  # 512 rows per core
P = 128
NSI = SH // P     # 4 si-tiles per core
NK = D // P       # 8 d_model tiles
NCH = S // 512    # 8 score chunks of 512 columns
NSJ = S // P      # 32 sj-tiles
SCALE = 1.0 / 8.0  # 1/sqrt(n_dim=64) per reference
LN_EPS = 1e-5
NEG = -1.0e30


def _emit(nc, tc, io):
    with ExitStack() as es:
        # ---------- long-lived pools (stack-ordered) ----------
        cpool = es.enter_context(tc.tile_pool(name="consts", bufs=1))
        xst_pool = es.enter_context(tc.tile_pool(name="xst", bufs=1))
        st_pool = es.enter_context(tc.tile_pool(name="stats", bufs=2))
        dram_pool = es.enter_context(
            tc.tile_pool(name="dramp", bufs=1, space="DRAM"))

        # internal DRAM for the K^T / V all-gathers (pool tiles => dep-tracked)
        # p-major layouts: one contiguous run per partition per chunk, so a
        # 2MB stream DMA is 128 descriptors instead of 1024.  K^T (f32r) and
        # V (bf16; its rounding error is diluted ~100x by the residual) share
        # ONE bounce buffer and ONE all-gather: the collective queue is
        # serial, so two gathers would chain their ~90us latencies.
        KCOLS = 2 * NK * SH          # K^T occupies 8192 bf16 slots (f32r x4096)
        VCOLS = 2 * NSI * 512        # V occupies 4096 bf16 slots
        kv_sh = dram_pool.tile([P, KCOLS + VCOLS], bf, tag="kv_sh")
        kv_full = dram_pool.tile([NCORES, P, KCOLS + VCOLS], bf, tag="kv_full",
                                 addr_space="Shared")

        # dummy collective: absorbs the one-time collective-mesh startup
        # latency concurrently with the compute prologue
        dumm_in = dram_pool.tile([1, 16], f32, tag="dumm_in")
        dumm_out = dram_pool.tile([NCORES, 16], f32, tag="dumm_out")
        with tc.high_priority():
            nc.sync.dma_start(dumm_in[:], io["ones_dram"].ap()[0:1, 0:16])
            nc.gpsimd.collective_compute(
                "AllGather", OP.bypass,
                replica_groups=[list(range(NCORES))],
                ins=[dumm_in.opt()], outs=[dumm_out.opt()],
            )

        # constants
        ident32 = cpool.tile([P, P], f32, tag="id32")
        identr = cpool.tile([P, P], f32r, tag="idr")
        nc.sync.dma_start(ident32[:], io["ident_dram"].ap())
        nc.sync.dma_start(identr[:], io["ident_dram"].ap().bitcast(f32r))
        onesr = cpool.tile([1, SH], f32r, tag="onesr")
        nc.sync.dma_start(onesr[:], io["ones_dram"].ap().bitcast(f32r))
        ones_l = onesr[0:1, 0:P]     # K=1 stationary operand for rank-1 bias
        bvec = {}
        for n in ("bq", "bk", "bv", "bo", "gamma", "beta"):
            bvec[n] = cpool.tile([1, D], f32r, tag=n, name="bv_" + n)
            nc.sync.dma_start(bvec[n][:], io[n + "_d"].bitcast(f32r))

        # broadcast gamma/beta to [P, D] via rank-1 matmuls
        gamma_b = cpool.tile([P, D], bf, tag="gammab")
        beta_b = cpool.tile([P, D], bf, tag="betab")
        with tc.tile_pool(name="psgb", bufs=2, space="PSUM") as psgb:
            for half in range(2):
                pg = psgb.tile([P, 512], f32, tag="pg")
                nc.tensor.matmul(pg[:], ones_l, bvec["gamma"][0:1, ts(half, 512)],
                                 start=True, stop=True)
                nc.vector.tensor_copy(gamma_b[:, ts(half, 512)], pg[:])
                pb = psgb.tile([P, 512], f32, tag="pb")
                nc.tensor.matmul(pb[:], ones_l, bvec["beta"][0:1, ts(half, 512)],
                                 start=True, stop=True)
                nc.vector.tensor_copy(beta_b[:, ts(half, 512)], pb[:])

        # persistent SBUF tensors
        xstr = [xst_pool.tile([P, SH], f32r, tag=f"xstr_{k}", name=f"xstr_{k}") for k in range(NK)]
        sums = [st_pool.tile([P, NCH], f32, tag=f"sums{si}", name=f"sums{si}") for si in range(NSI)]
        r_bcast = st_pool.tile([P, SH], f32, tag="r_bcast", name="r_bcast")
        rrec = [st_pool.tile([P, 1], f32, tag=f"rrec{si}", name=f"rrec{si}") for si in range(NSI)]

        # ---------- phase T: load X shard, transpose ----------
        with (
            nc.named_scope("phaseT"),
            tc.tile_pool(name="xs", bufs=1) as xs_pool,
            tc.tile_pool(name="psx", bufs=2, space="PSUM") as psx_pool,
        ):
            xs = [xs_pool.tile([P, D], f32, tag=f"xs{si}", name=f"xs{si}") for si in range(NSI)]
            for si in range(NSI):
                nc.sync.dma_start(xs[si][:], io["x_d"][ts(si, P), :])
            for k in range(NK):
                psx = psx_pool.tile([P, SH], f32, tag="psx")
                for si in range(NSI):
                    nc.tensor.transpose(psx[:, ts(si, P)], xs[si][:, ts(k, P)],
                                        ident32[:])
                nc.vector.tensor_copy(xstr[k][:], psx[:])

        # ---------- phase QKV ----------
        qt_pool = es.enter_context(tc.tile_pool(name="qtpool", bufs=1))
        qt_t = [qt_pool.tile([P, SH], f32r, tag=f"qt{m}", name=f"qtt{m}")
                for m in range(NK)]
        with (
            nc.named_scope("qkv"),
            tc.tile_pool(name="wsl", bufs=16) as w_pool,
            tc.tile_pool(name="ktloc", bufs=1) as ktl_pool,
            tc.tile_pool(name="vsb", bufs=1) as v_sb_pool,
            tc.tile_pool(name="psqk", bufs=3, space="PSUM") as psqk_pool,
        ):
            # K^T then V, then ONE combined all-gather overlapping Q compute.
            # Weights DMA'd as full [128, 1024] rows (4KB/partition) --
            # small column slices would run the queues at ~40% efficiency.
            ktbig = ktl_pool.tile([P, NK * SH], f32r, tag="ktbig",
                                  name="ktbig")
            ktloc = [ktbig[:, ts(m, SH)] for m in range(NK)]
            wkt = []
            for k in range(NK):
                w1 = w_pool.tile([P, D], f32r, tag="wfull")
                nc.sync.dma_start(w1[:], io["wk_d"][ts(k, P), :].bitcast(f32r))
                wkt.append(w1)
            for m in range(NK):
                ps = psqk_pool.tile([P, SH], f32, tag="psqk")
                for k in range(NK):
                    nc.tensor.matmul(ps[:], wkt[k][:, ts(m, P)], xstr[k][:],
                                     start=(k == 0), stop=False)
                nc.tensor.matmul(ps[:], bvec["bk"][0:1, ts(m, P)], onesr[:],
                                 start=False, stop=True)
                with tc.high_priority():
                    nc.vector.tensor_copy(ktloc[m], ps[:])
            with tc.high_priority():
                nc.scalar.dma_start(kv_sh[:, 0:KCOLS].bitcast(f32r), ktbig[:])
            # V in natural layout: out[si-tile, n] = X @ Wv  (bf16 bounce)
            wvt = []
            for k in range(NK):
                w1 = w_pool.tile([P, D], f32r, tag="wfull")
                nc.sync.dma_start(w1[:], io["wv_d"][ts(k, P), :].bitcast(f32r))
                wvt.append(w1)
            vbig = v_sb_pool.tile([P, NSI * 2 * 512], bf, tag="vbig",
                                  name="vbig")
            for si in range(NSI):
                for nh in range(2):
                    ps = psqk_pool.tile([P, SH], f32, tag="psqk")
                    for k in range(NK):
                        nc.tensor.matmul(ps[:], xstr[k][:, ts(si, P)],
                                         wvt[k][:, ts(nh, 512)],
                                         start=(k == 0), stop=False)
                    nc.tensor.matmul(ps[:], ones_l, bvec["bv"][0:1, ts(nh, 512)],
                                     start=False, stop=True)
                    with tc.high_priority():
                        nc.vector.tensor_copy(vbig[:, ts(nh * NSI + si, 512)],
                                              ps[:])
            with tc.high_priority():
                nc.scalar.dma_start(kv_sh[:, KCOLS:], vbig[:])
                nc.gpsimd.collective_compute(
                    "AllGather", OP.bypass,
                    replica_groups=[list(range(NCORES))],
                    ins=[kv_sh.opt()], outs=[kv_full.opt()],
                )
            # Q^T last: overlaps the gather
            wqt = []
            for k in range(NK):
                w1 = w_pool.tile([P, D], f32r, tag="wfull")
                nc.sync.dma_start(w1[:], io["wq_d"][ts(k, P), :].bitcast(f32r))
                wqt.append(w1)
            for m in range(NK):
                ps = psqk_pool.tile([P, SH], f32, tag="psqk")
                for k in range(NK):
                    nc.tensor.matmul(ps[:], wqt[k][:, ts(m, P)], xstr[k][:],
                                     start=(k == 0), stop=False)
                nc.tensor.matmul(ps[:], bvec["bq"][0:1, ts(m, P)], onesr[:],
                                 start=False, stop=True)
                nc.vector.tensor_copy(qt_t[m][:], ps[:])

        # ---------- phase scores + softmax ----------
        a_pool = es.enter_context(tc.tile_pool(name="apool", bufs=1))
        a_t = [a_pool.tile([P, S], f32r, tag=f"a{si}", name=f"a{si}")
               for si in range(NSI)]
        masks = []
        with (
            nc.named_scope("scores"),
            tc.tile_pool(name="ktst", bufs=3) as kt_pool,
            tc.tile_pool(name="mrot", bufs=1) as m_pool,
            tc.tile_pool(name="pss", bufs=1, space="PSUM") as pss_pool,
        ):
            negc_b = m_pool.tile([P, 512], f32, tag="negc")
            nc.vector.memset(negc_b[:], NEG / SCALE)
            for c in range(NCH):
                # one batched DMA for the whole K^T chunk (8 m-tiles)
                kt = kt_pool.tile([P, NK * SH], f32r, tag="kt")
                eng = nc.scalar if c % 2 == 0 else nc.sync
                eng.dma_start(kt[:], kv_full[c][:, 0:KCOLS].bitcast(f32r))
                if c == 0:
                    for si in range(NSI):
                        m1 = m_pool.tile([P, S], u8, tag=f"mask{si}",
                                         name=f"mask{si}")
                        nc.sync.dma_start(m1[:], io["mask_d"][ts(si, P), :])
                        masks.append(m1)
                for si in range(NSI):
                    ps = pss_pool.tile([P, 512], f32, tag=f"pss{si}")
                    for m in range(NK):
                        nc.tensor.matmul(ps[:], qt_t[m][:, ts(si, P)],
                                         kt[:, ts(m, SH)],
                                         start=(m == 0), stop=(m == NK - 1))
                    # masked scores -> -1e30; then exp(SCALE * x)
                    nc.vector.copy_predicated(ps[:], masks[si][:, ts(c, 512)],
                                              negc_b[:])
                    nc.scalar.activation(
                        a_t[si][:, ts(c, 512)], ps[:], AF.Exp,
                        scale=SCALE, accum_out=sums[si][:, c:c + 1])
            # row-sum reciprocals; build r broadcast [P, 512] along the
            # si free dim (rank-1) so C^T can be scaled without normalizing
            # A first -- the AV transposes then need only the raw exp values.
            for si in range(NSI):
                lsum = st_pool.tile([P, 1], f32, tag="lsum")
                nc.vector.tensor_reduce(lsum[:], sums[si][:], AX.X, OP.add)
                nc.vector.reciprocal(rrec[si][:], lsum[:])
            r4 = st_pool.tile([P, 4], f32, tag="r4")
            for si in range(NSI):
                nc.vector.tensor_copy(r4[:, si:si + 1], rrec[si][:])
            ps_r = pss_pool.tile([P, 512], f32, tag="pss0")
            nc.tensor.transpose(ps_r[0:4, 0:P], r4[:], ident32[:])
            r4t = st_pool.tile([4, P], f32, tag="r4t")
            nc.vector.tensor_copy(r4t[:], ps_r[0:4, 0:P])
            r_row = st_pool.tile([1, SH], f32r, tag="r_row")
            for si in range(NSI):
                nc.sync.dma_start(r_row[0:1, ts(si, P)].bitcast(f32),
                                  r4t[si:si + 1, :])
            ps_rb = pss_pool.tile([P, 512], f32, tag="pss1")
            nc.tensor.matmul(ps_rb[:], ones_l, r_row[:], start=True, stop=True)
            nc.vector.tensor_copy(r_bcast[:], ps_rb[:])

        # ---------- phase AV: C^T += V^T slice @ A^T, 2 column halves ----------
        ct_pool = es.enter_context(tc.tile_pool(name="ctpool", bufs=1))
        ct_t = [ct_pool.tile([P, SH], f32r, tag=f"ct{k}", name=f"ctt{k}")
                for k in range(NK)]
        wo_pool = es.enter_context(tc.tile_pool(name="wo", bufs=1))
        wo_sl = []
        for nh in range(2):
            for k in range(NK):
                w1 = wo_pool.tile([P, 512], f32r, tag=f"wo{nh}_{k}",
                                  name=f"wo{nh}_{k}")
                nc.sync.dma_start(
                    w1[:], io["wo_d"][ts(k, P), ts(nh, 512)].bitcast(f32r))
                wo_sl.append(w1)
        with (
            nc.named_scope("av"),
            tc.tile_pool(name="etrot", bufs=2) as et_pool,
            tc.tile_pool(name="vrot", bufs=3) as vr_pool,
            tc.tile_pool(name="psct", bufs=1, space="PSUM") as psct_pool,
            tc.tile_pool(name="pstr", bufs=2, space="PSUM") as pstr_pool,
        ):
            for h in range(2):
                psct = [psct_pool.tile([P, SH], f32, tag=f"psct{nt}", name=f"psct{nt}")
                        for nt in range(4)]
                for sj in range(NSJ):
                    if sj % 4 == 0:
                        # batched V DMA: this core-chunk's half (4 sj-tiles)
                        vt = vr_pool.tile([P, 4 * 512], bf, tag="vt")
                        veng = nc.scalar if (sj // 4) % 2 == 0 else nc.sync
                        veng.dma_start(
                            vt[:],
                            kv_full[sj // 4][:, KCOLS + h * 2048:
                                             KCOLS + (h + 1) * 2048])
                    vs = vt[:, ts(sj % 4, 512)]
                    pst = pstr_pool.tile([P, SH], f32r, tag="pstr")
                    for si in range(NSI):
                        nc.tensor.transpose(pst[:, ts(si, P)],
                                            a_t[si][:, ts(sj, P)], identr[:])
                    ett = et_pool.tile([P, SH], bf, tag="ett")
                    nc.vector.tensor_copy(ett[:], pst[:].bitcast(f32))
                    for nt in range(4):
                        nc.tensor.matmul(psct[nt][:],
                                         vs[:, ts(nt, P)], ett[:],
                                         start=(sj == 0), stop=(sj == NSJ - 1))
                for nt in range(4):
                    k = h * 4 + nt
                    # C^T = r * (E@V)^T + X^T  (softmax scale + residual)
                    nc.vector.tensor_mul(psct[nt][:], psct[nt][:], r_bcast[:])
                    nc.vector.tensor_add(ct_t[k][:], psct[nt][:], xstr[k][:])

        # normalize A for the attention output (runs during/after AV; the
        # WAR on the AV transposes orders it automatically)
        for si in range(NSI):
            nc.vector.tensor_scalar_mul(a_t[si][:], a_t[si][:], rrec[si][:])
            nc.gpsimd.dma_start(io["att_d"][ts(si, P), :],
                                a_t[si][:].bitcast(f32))

        # ---------- phase out-proj + LayerNorm ----------
        with (
            nc.named_scope("hln"),
            tc.tile_pool(name="hc", bufs=2) as hc_pool,
            tc.tile_pool(name="osb", bufs=2) as o_pool,
            tc.tile_pool(name="scr", bufs=2) as scr_pool,
            tc.tile_pool(name="psh", bufs=1, space="PSUM") as psh_pool,
        ):
            psh = [[psh_pool.tile([P, 512], f32, tag=f"psh{si}_{nh}", name=f"psh{si}_{nh}")
                    for nh in range(2)] for si in range(NSI)]
            inv_d = 1.0 / D
            for shalf in range(2):
                sis = (shalf * 2, shalf * 2 + 1)
                for nh in range(2):
                    for k in range(NK):
                        for si in sis:
                            nc.tensor.matmul(psh[si][nh][:], ct_t[k][:, ts(si, P)],
                                             wo_sl[nh * NK + k][:],
                                             start=(k == 0), stop=False)
                    for si in sis:
                        nc.tensor.matmul(psh[si][nh][:], ones_l,
                                         bvec["bo"][0:1, ts(nh, 512)],
                                         start=False, stop=True)
            for si in range(NSI):
                st2 = st_pool.tile([P, 2], f32, tag="st2")
                for nh in range(2):
                    nc.vector.tensor_reduce(st2[:, nh:nh + 1], psh[si][nh][:],
                                            AX.X, OP.add)
                mu = st_pool.tile([P, 1], f32, tag="mu")
                nc.vector.tensor_add(mu[:], st2[:, 0:1], st2[:, 1:2])
                nc.vector.tensor_scalar_mul(mu[:], mu[:], inv_d)
                hc = [hc_pool.tile([P, 512], f32, tag=f"hc{nh}", name=f"hc{nh}")
                      for nh in range(2)]
                ss2 = st_pool.tile([P, 2], f32, tag="ss2")
                for nh in range(2):
                    nc.vector.tensor_scalar(hc[nh][:], psh[si][nh][:], mu[:],
                                            None, op0=OP.subtract)
                    scr = scr_pool.tile([P, 512], f32, tag="scr")
                    nc.scalar.activation(scr[:], hc[nh][:], AF.Square,
                                         accum_out=ss2[:, nh:nh + 1])
                var = st_pool.tile([P, 1], f32, tag="var")
                nc.vector.tensor_add(var[:], ss2[:, 0:1], ss2[:, 1:2])
                nc.vector.tensor_scalar(var[:], var[:], inv_d, LN_EPS,
                                        op0=OP.mult, op1=OP.add)
                rvar = st_pool.tile([P, 1], f32, tag="rvar")
                nc.vector.reciprocal(rvar[:], var[:])
                rln = st_pool.tile([P, 1], f32, tag="rln")
                nc.scalar.activation(rln[:], rvar[:], AF.Sqrt)
                osb = o_pool.tile([P, D], f32, tag="osb")
                for nh in range(2):
                    nc.vector.scalar_tensor_tensor(
                        osb[:, ts(nh, 512)], hc[nh][:], rln[:],
                        gamma_b[:, ts(nh, 512)], op0=OP.mult, op1=OP.mult)
                    nc.vector.tensor_add(osb[:, ts(nh, 512)],
                                         osb[:, ts(nh, 512)],
                                         beta_b[:, ts(nh, 512)])
                nc.gpsimd.dma_start(io["out_d"][ts(si, P), :], osb[:])


def build_program():
    nc = bacc.Bacc("TRN2", target_bir_lowering=False, debug=False,
                   num_devices=NCORES)
    io = {
        "x_d": nc.dram_tensor("x", [SH, D], f32, kind="ExternalInput").ap(),
        "mask_d": nc.dram_tensor("mask", [SH, S], u8, kind="ExternalInput").ap(),
        "wq_d": nc.dram_tensor("wq", [D, D], f32, kind="ExternalInput").ap(),
        "wk_d": nc.dram_tensor("wk", [D, D], f32, kind="ExternalInput").ap(),
        "wv_d": nc.dram_tensor("wv", [D, D], f32, kind="ExternalInput").ap(),
        "wo_d": nc.dram_tensor("wo", [D, D], f32, kind="ExternalInput").ap(),
        "bq_d": nc.dram_tensor("bq", [1, D], f32, kind="ExternalInput").ap(),
        "bk_d": nc.dram_tensor("bk", [1, D], f32, kind="ExternalInput").ap(),
        "bv_d": nc.dram_tensor("bv", [1, D], f32, kind="ExternalInput").ap(),
        "bo_d": nc.dram_tensor("bo", [1, D], f32, kind="ExternalInput").ap(),
        "gamma_d": nc.dram_tensor("gamma", [1, D], f32,
                                  kind="ExternalInput").ap(),
        "beta_d": nc.dram_tensor("beta", [1, D], f32, kind="ExternalInput").ap(),
        "att_d": nc.dram_tensor("attn_out", [SH, S], f32,
                                kind="ExternalOutput").ap(),
        "out_d": nc.dram_tensor("out", [SH, D], f32, kind="ExternalOutput").ap(),
        "ident_dram": nc.inline_tensor(np.eye(P, dtype=np.float32),
                                       name="ident_c"),
        "ones_dram": nc.inline_tensor(np.ones((1, SH), dtype=np.float32),
                                      name="ones_c"),
    }
    with tile.TileContext(nc) as tc:
        _emit(nc, tc, io)
    nc.compile()
    return nc


_CACHED = None


def _get_program():
    global _CACHED
    if _CACHED is None:
        _CACHED = build_program()
    return _CACHED


def _make_in_maps(embed_word_vec, attn_mask, wq, bq, wk, bk, wv, bv, wo, bo,
                  gamma, beta):
    x = np.ascontiguousarray(np.asarray(embed_word_vec, dtype=np.float32))
    mask = np.ascontiguousarray(np.asarray(attn_mask)).view(np.uint8)
    w_ = {n: np.ascontiguousarray(np.asarray(v, dtype=np.float32))
          for n, v in (("wq", wq), ("wk", wk), ("wv", wv), ("wo", wo))}
    b_ = {n: np.ascontiguousarray(
              np.asarray(v, dtype=np.float32)).reshape(1, D)
          for n, v in (("bq", bq), ("bk", bk), ("bv", bv), ("bo", bo),
                       ("gamma", gamma), ("beta", beta))}
    in_maps = []
    for c in range(NCORES):
        m = {"x": x[c * SH:(c + 1) * SH], "mask": mask[c * SH:(c + 1) * SH]}
        m.update(w_)
        m.update(b_)
        in_maps.append(m)
    return in_maps


def _assemble(res):
    attention = np.concatenate(
        [res.results[c]["attn_out"] for c in range(NCORES)], axis=0)
    output = np.concatenate(
        [res.results[c]["out"] for c in range(NCORES)], axis=0)
    return (output, attention)


def kernel(embed_word_vec, attn_mask, wq, bq, wk, bk, wv, bv, wo, bo, gamma,
           beta):
    nc = _get_program()
    in_maps = _make_in_maps(embed_word_vec, attn_mask, wq, bq, wk, bk, wv, bv,
                            wo, bo, gamma, beta)
    res = run_bass_kernel_spmd(nc, in_maps, list(range(NCORES)))
    return _assemble(res)


# revision 33
# speedup vs baseline: 1.0357x; 1.0357x over previous
"""Fused attention block (QKV -> masked softmax -> AV+residual -> out-proj -> LN)
for one trn2 chip (8 NeuronCores), sequence-parallel over the 4096 rows.

Self-contained: hardcodes shapes/sharding; builds+compiles the Bass program on
first call and caches it.
"""
import sys

sys.path.insert(0, "/opt/trn_rl_repo")

from contextlib import ExitStack  # noqa: E402

import numpy as np  # noqa: E402

import concourse.bacc as bacc  # noqa: E402
import concourse.bass as bass  # noqa: E402
import concourse.tile as tile  # noqa: E402
from concourse import mybir  # noqa: E402
from concourse.bass_utils import run_bass_kernel_spmd  # noqa: E402

f32 = mybir.dt.float32
f32r = mybir.dt.float32r
u8 = mybir.dt.uint8
bf = mybir.dt.bfloat16
AF = mybir.ActivationFunctionType
OP = mybir.AluOpType
AX = mybir.AxisListType
ts = bass.ts

NCORES = 8
S = 4096          # full sequence
D = 1024          # d_model
SH = S // NCORES  # 512 rows per core
P = 128
NSI = SH // P     # 4 si-tiles per core
NK = D // P       # 8 d_model tiles
NCH = S // 512    # 8 score chunks of 512 columns
NSJ = S // P      # 32 sj-tiles
SCALE = 1.0 / 8.0  # 1/sqrt(n_dim=64) per reference
LN_EPS = 1e-5
NEG = -1.0e30


def _emit(nc, tc, io):
    with ExitStack() as es:
        # ---------- long-lived pools (stack-ordered) ----------
        cpool = es.enter_context(tc.tile_pool(name="consts", bufs=1))
        xst_pool = es.enter_context(tc.tile_pool(name="xst", bufs=1))
        st_pool = es.enter_context(tc.tile_pool(name="stats", bufs=2))
        dram_pool = es.enter_context(
            tc.tile_pool(name="dramp", bufs=1, space="DRAM"))

        # internal DRAM for the K^T / V all-gathers (pool tiles => dep-tracked)
        # p-major layouts: one contiguous run per partition per chunk, so a
        # 2MB stream DMA is 128 descriptors instead of 1024.  K^T (f32r) and
        # V (bf16; its rounding error is diluted ~100x by the residual) share
        # ONE bounce buffer and ONE all-gather: the collective queue is
        # serial, so two gathers would chain their ~90us latencies.
        KCOLS = 2 * NK * SH          # K^T occupies 8192 bf16 slots (f32r x4096)
        VCOLS = 2 * NSI * 512        # V occupies 4096 bf16 slots
        kv_sh = dram_pool.tile([P, KCOLS + VCOLS], bf, tag="kv_sh")
        kv_full = dram_pool.tile([NCORES, P, KCOLS + VCOLS], bf, tag="kv_full",
                                 addr_space="Shared")

        # dummy collective: absorbs the one-time collective-mesh startup
        # latency concurrently with the compute prologue
        dumm_in = dram_pool.tile([1, 16], f32, tag="dumm_in")
        dumm_out = dram_pool.tile([NCORES, 16], f32, tag="dumm_out")
        with tc.high_priority():
            nc.sync.dma_start(dumm_in[:], io["ones_dram"].ap()[0:1, 0:16])
            nc.gpsimd.collective_compute(
                "AllGather", OP.bypass,
                replica_groups=[list(range(NCORES))],
                ins=[dumm_in.opt()], outs=[dumm_out.opt()],
            )

        # constants
        ident32 = cpool.tile([P, P], f32, tag="id32")
        identr = cpool.tile([P, P], f32r, tag="idr")
        nc.sync.dma_start(ident32[:], io["ident_dram"].ap())
        nc.sync.dma_start(identr[:], io["ident_dram"].ap().bitcast(f32r))
        onesr = cpool.tile([1, SH], f32r, tag="onesr")
        nc.sync.dma_start(onesr[:], io["ones_dram"].ap().bitcast(f32r))
        ones_l = onesr[0:1, 0:P]     # K=1 stationary operand for rank-1 bias
        bvec = {}
        for n in ("bq", "bk", "bv", "bo", "gamma", "beta"):
            bvec[n] = cpool.tile([1, D], f32r, tag=n, name="bv_" + n)
            nc.sync.dma_start(bvec[n][:], io[n + "_d"].bitcast(f32r))

        # broadcast gamma/beta to [P, D] via rank-1 matmuls
        gamma_b = cpool.tile([P, D], bf, tag="gammab")
        beta_b = cpool.tile([P, D], bf, tag="betab")
        with tc.tile_pool(name="psgb", bufs=2, space="PSUM") as psgb:
            for half in range(2):
                pg = psgb.tile([P, 512], f32, tag="pg")
                nc.tensor.matmul(pg[:], ones_l, bvec["gamma"][0:1, ts(half, 512)],
                                 start=True, stop=True)
                nc.vector.tensor_copy(gamma_b[:, ts(half, 512)], pg[:])
                pb = psgb.tile([P, 512], f32, tag="pb")
                nc.tensor.matmul(pb[:], ones_l, bvec["beta"][0:1, ts(half, 512)],
                                 start=True, stop=True)
                nc.vector.tensor_copy(beta_b[:, ts(half, 512)], pb[:])

        # persistent SBUF tensors
        xstr = [xst_pool.tile([P, SH], f32r, tag=f"xstr_{k}", name=f"xstr_{k}") for k in range(NK)]
        sums = [st_pool.tile([P, NCH], f32, tag=f"sums{si}", name=f"sums{si}") for si in range(NSI)]
        r_bcast = st_pool.tile([P, SH], f32, tag="r_bcast", name="r_bcast")
        rrec = [st_pool.tile([P, 1], f32, tag=f"rrec{si}", name=f"rrec{si}") for si in range(NSI)]

        # ---------- phase T: load X shard, transpose ----------
        with (
            nc.named_scope("phaseT"),
            tc.tile_pool(name="xs", bufs=1) as xs_pool,
            tc.tile_pool(name="psx", bufs=2, space="PSUM") as psx_pool,
        ):
            xs = [xs_pool.tile([P, D], f32, tag=f"xs{si}", name=f"xs{si}") for si in range(NSI)]
            for si in range(NSI):
                nc.sync.dma_start(xs[si][:], io["x_d"][ts(si, P), :])
            for k in range(NK):
                psx = psx_pool.tile([P, SH], f32, tag="psx")
                for si in range(NSI):
                    nc.tensor.transpose(psx[:, ts(si, P)], xs[si][:, ts(k, P)],
                                        ident32[:])
                nc.vector.tensor_copy(xstr[k][:], psx[:])

        # ---------- phase QKV ----------
        qt_pool = es.enter_context(tc.tile_pool(name="qtpool", bufs=1))
        qt_t = [qt_pool.tile([P, SH], f32r, tag=f"qt{m}", name=f"qtt{m}")
                for m in range(NK)]
        with (
            nc.named_scope("qkv"),
            tc.tile_pool(name="wsl", bufs=16) as w_pool,
            tc.tile_pool(name="ktloc", bufs=1) as ktl_pool,
            tc.tile_pool(name="vsb", bufs=1) as v_sb_pool,
            tc.tile_pool(name="psqk", bufs=3, space="PSUM") as psqk_pool,
        ):
            # K^T then V, then ONE combined all-gather overlapping Q compute.
            # Weights DMA'd as full [128, 1024] rows (4KB/partition) --
            # small column slices would run the queues at ~40% efficiency.
            ktbig = ktl_pool.tile([P, NK * SH], f32r, tag="ktbig",
                                  name="ktbig")
            ktloc = [ktbig[:, ts(m, SH)] for m in range(NK)]
            wkt = []
            for k in range(NK):
                w1 = w_pool.tile([P, D], f32r, tag="wfull")
                nc.sync.dma_start(w1[:], io["wk_d"][ts(k, P), :].bitcast(f32r))
                wkt.append(w1)
            for m in range(NK):
                ps = psqk_pool.tile([P, SH], f32, tag="psqk")
                for k in range(NK):
                    nc.tensor.matmul(ps[:], wkt[k][:, ts(m, P)], xstr[k][:],
                                     start=(k == 0), stop=False)
                nc.tensor.matmul(ps[:], bvec["bk"][0:1, ts(m, P)], onesr[:],
                                 start=False, stop=True)
                with tc.high_priority():
                    nc.vector.tensor_copy(ktloc[m], ps[:])
            with tc.high_priority():
                nc.scalar.dma_start(kv_sh[:, 0:KCOLS].bitcast(f32r), ktbig[:])
            # V in natural layout: out[si-tile, n] = X @ Wv  (bf16 bounce)
            wvt = []
            for k in range(NK):
                w1 = w_pool.tile([P, D], f32r, tag="wfull")
                nc.sync.dma_start(w1[:], io["wv_d"][ts(k, P), :].bitcast(f32r))
                wvt.append(w1)
            vbig = v_sb_pool.tile([P, NSI * 2 * 512], bf, tag="vbig",
                                  name="vbig")
            for si in range(NSI):
                for nh in range(2):
                    ps = psqk_pool.tile([P, SH], f32, tag="psqk")
                    for k in range(NK):
                        nc.tensor.matmul(ps[:], xstr[k][:, ts(si, P)],
                                         wvt[k][:, ts(nh, 512)],
                                         start=(k == 0), stop=False)
                    nc.tensor.matmul(ps[:], ones_l, bvec["bv"][0:1, ts(nh, 512)],
                                     start=False, stop=True)
                    with tc.high_priority():
                        nc.vector.tensor_copy(vbig[:, ts(nh * NSI + si, 512)],
                                              ps[:])
            with tc.high_priority():
                nc.scalar.dma_start(kv_sh[:, KCOLS:], vbig[:])
                nc.gpsimd.collective_compute(
                    "AllGather", OP.bypass,
                    replica_groups=[list(range(NCORES))],
                    ins=[kv_sh.opt()], outs=[kv_full.opt()],
                )
            # Q^T last: overlaps the gather
            wqt = []
            for k in range(NK):
                w1 = w_pool.tile([P, D], f32r, tag="wfull")
                nc.sync.dma_start(w1[:], io["wq_d"][ts(k, P), :].bitcast(f32r))
                wqt.append(w1)
            for m in range(NK):
                ps = psqk_pool.tile([P, SH], f32, tag="psqk")
                for k in range(NK):
                    nc.tensor.matmul(ps[:], wqt[k][:, ts(m, P)], xstr[k][:],
                                     start=(k == 0), stop=False)
                nc.tensor.matmul(ps[:], bvec["bq"][0:1, ts(m, P)], onesr[:],
                                 start=False, stop=True)
                nc.vector.tensor_copy(qt_t[m][:], ps[:])

        # ---------- phase scores + softmax ----------
        a_pool = es.enter_context(tc.tile_pool(name="apool", bufs=1))
        a_t = [a_pool.tile([P, S], f32r, tag=f"a{si}", name=f"a{si}")
               for si in range(NSI)]
        masks = []
        with (
            nc.named_scope("scores"),
            tc.tile_pool(name="ktst", bufs=3) as kt_pool,
            tc.tile_pool(name="mrot", bufs=1) as m_pool,
            tc.tile_pool(name="pss", bufs=1, space="PSUM") as pss_pool,
        ):
            negc_b = m_pool.tile([P, 512], f32, tag="negc")
            nc.vector.memset(negc_b[:], NEG / SCALE)
            for c in range(NCH):
                # one batched DMA for the whole K^T chunk (8 m-tiles)
                kt = kt_pool.tile([P, NK * SH], f32r, tag="kt")
                eng = nc.scalar if c % 2 == 0 else nc.sync
                eng.dma_start(kt[:], kv_full[c][:, 0:KCOLS].bitcast(f32r))
                if c == 0:
                    for si in range(NSI):
                        m1 = m_pool.tile([P, S], u8, tag=f"mask{si}",
                                         name=f"mask{si}")
                        nc.sync.dma_start(m1[:], io["mask_d"][ts(si, P), :])
                        masks.append(m1)
                for si in range(NSI):
                    ps = pss_pool.tile([P, 512], f32, tag=f"pss{si}")
                    for m in range(NK):
                        nc.tensor.matmul(ps[:], qt_t[m][:, ts(si, P)],
                                         kt[:, ts(m, SH)],
                                         start=(m == 0), stop=(m == NK - 1))
                    # masked scores -> -1e30; then exp(SCALE * x)
                    nc.vector.copy_predicated(ps[:], masks[si][:, ts(c, 512)],
                                              negc_b[:])
                    nc.scalar.activation(
                        a_t[si][:, ts(c, 512)], ps[:], AF.Exp,
                        scale=SCALE, accum_out=sums[si][:, c:c + 1])
            # row-sum reciprocals; build r broadcast [P, 512] along the
            # si free dim (rank-1) so C^T can be scaled without normalizing
            # A first -- the AV transposes then need only the raw exp values.
            for si in range(NSI):
                lsum = st_pool.tile([P, 1], f32, tag="lsum")
                nc.vector.tensor_reduce(lsum[:], sums[si][:], AX.X, OP.add)
                nc.vector.reciprocal(rrec[si][:], lsum[:])
            r4 = st_pool.tile([P, 4], f32, tag="r4")
            for si in range(NSI):
                nc.vector.tensor_copy(r4[:, si:si + 1], rrec[si][:])
            ps_r = pss_pool.tile([P, 512], f32, tag="pss0")
            nc.tensor.transpose(ps_r[0:4, 0:P], r4[:], ident32[:])
            r4t = st_pool.tile([4, P], f32, tag="r4t")
            nc.vector.tensor_copy(r4t[:], ps_r[0:4, 0:P])
            r_row = st_pool.tile([1, SH], f32r, tag="r_row")
            for si in range(NSI):
                nc.sync.dma_start(r_row[0:1, ts(si, P)].bitcast(f32),
                                  r4t[si:si + 1, :])
            ps_rb = pss_pool.tile([P, 512], f32, tag="pss1")
            nc.tensor.matmul(ps_rb[:], ones_l, r_row[:], start=True, stop=True)
            nc.vector.tensor_copy(r_bcast[:], ps_rb[:])

        # ---------- phase AV: C^T += V^T slice @ A^T, 2 column halves ----------
        ct_pool = es.enter_context(tc.tile_pool(name="ctpool", bufs=1))
        ct_t = [ct_pool.tile([P, SH], f32r, tag=f"ct{k}", name=f"ctt{k}")
                for k in range(NK)]
        wo_pool = es.enter_context(tc.tile_pool(name="wo", bufs=1))
        wo_sl = []
        for nh in range(2):
            for k in range(NK):
                w1 = wo_pool.tile([P, 512], f32r, tag=f"wo{nh}_{k}",
                                  name=f"wo{nh}_{k}")
                nc.sync.dma_start(
                    w1[:], io["wo_d"][ts(k, P), ts(nh, 512)].bitcast(f32r))
                wo_sl.append(w1)
        with (
            nc.named_scope("av"),
            tc.tile_pool(name="etrot", bufs=2) as et_pool,
            tc.tile_pool(name="vrot", bufs=3) as vr_pool,
            tc.tile_pool(name="psct", bufs=1, space="PSUM") as psct_pool,
            tc.tile_pool(name="pstr", bufs=2, space="PSUM") as pstr_pool,
        ):
            for h in range(2):
                psct = [psct_pool.tile([P, SH], f32, tag=f"psct{nt}", name=f"psct{nt}")
                        for nt in range(4)]
                for sj in range(NSJ):
                    if sj % 4 == 0:
                        # batched V DMA: this core-chunk's half (4 sj-tiles)
                        vt = vr_pool.tile([P, 4 * 512], bf, tag="vt")
                        veng = nc.scalar if (sj // 4) % 2 == 0 else nc.sync
                        veng.dma_start(
                            vt[:],
                            kv_full[sj // 4][:, KCOLS + h * 2048:
                                             KCOLS + (h + 1) * 2048])
                    vs = vt[:, ts(sj % 4, 512)]
                    pst = pstr_pool.tile([P, SH], f32r, tag="pstr")
                    for si in range(NSI):
                        nc.tensor.transpose(pst[:, ts(si, P)],
                                            a_t[si][:, ts(sj, P)], identr[:])
                    ett = et_pool.tile([P, SH], bf, tag="ett")
                    nc.vector.tensor_copy(ett[:], pst[:].bitcast(f32))
                    for nt in range(4):
                        nc.tensor.matmul(psct[nt][:],
                                         vs[:, ts(nt, P)], ett[:],
                                         start=(sj == 0), stop=(sj == NSJ - 1))
                for nt in range(4):
                    k = h * 4 + nt
                    # C^T = r * (E@V)^T + X^T  (softmax scale + residual)
                    nc.vector.tensor_mul(psct[nt][:], psct[nt][:], r_bcast[:])
                    nc.vector.tensor_add(ct_t[k][:], psct[nt][:], xstr[k][:])

        # normalize A for the attention output (runs during/after AV; the
        # WAR on the AV transposes orders it automatically)
        for si in range(NSI):
            nc.vector.tensor_scalar_mul(a_t[si][:], a_t[si][:], rrec[si][:])
            nc.gpsimd.dma_start(io["att_d"][ts(si, P), :],
                                a_t[si][:].bitcast(f32))

        # ---------- phase out-proj + LayerNorm ----------
        with (
            nc.named_scope("hln"),
            tc.tile_pool(name="hc", bufs=2) as hc_pool,
            tc.tile_pool(name="osb", bufs=2) as o_pool,
            tc.tile_pool(name="scr", bufs=2) as scr_pool,
            tc.tile_pool(name="psh", bufs=1, space="PSUM") as psh_pool,
        ):
            psh = [[psh_pool.tile([P, 512], f32, tag=f"psh{si}_{nh}", name=f"psh{si}_{nh}")
                    for nh in range(2)] for si in range(NSI)]
            inv_d = 1.0 / D
            for shalf in range(2):
                sis = (shalf * 2, shalf * 2 + 1)
                for nh in range(2):
                    for k in range(NK):
                        for si in sis:
                            nc.tensor.matmul(psh[si][nh][:], ct_t[k][:, ts(si, P)],
                                             wo_sl[nh * NK + k][:],
                                             start=(k == 0), stop=False)
                    for si in sis:
                        nc.tensor.matmul(psh[si][nh][:], ones_l,
                                         bvec["bo"][0:1, ts(nh, 512)],
                                         start=False, stop=True)
            for si in range(NSI):
                st2 = st_pool.tile([P, 2], f32, tag="st2")
                for nh in range(2):
                    nc.vector.tensor_reduce(st2[:, nh:nh + 1], psh[si][nh][:],
                                            AX.X, OP.add)
                mu = st_pool.tile([P, 1], f32, tag="mu")
                nc.vector.tensor_add(mu[:], st2[:, 0:1], st2[:, 1:2])
                nc.vector.tensor_scalar_mul(mu[:], mu[:], inv_d)
                hc = [hc_pool.tile([P, 512], f32, tag=f"hc{nh}", name=f"hc{nh}")
                      for nh in range(2)]
                ss2 = st_pool.tile([P, 2], f32, tag="ss2")
                for nh in range(2):
                    nc.vector.tensor_scalar(hc[nh][:], psh[si][nh][:], mu[:],
                                            None, op0=OP.subtract)
                    scr = scr_pool.tile([P, 512], f32, tag="scr")
                    nc.scalar.activation(scr[:], hc[nh][:], AF.Square,
                                         accum_out=ss2[:, nh:nh + 1])
                var = st_pool.tile([P, 1], f32, tag="var")
                nc.vector.tensor_add(var[:], ss2[:, 0:1], ss2[:, 1:2])
                nc.vector.tensor_scalar(var[:], var[:], inv_d, LN_EPS,
                                        op0=OP.mult, op1=OP.add)
                rvar = st_pool.tile([P, 1], f32, tag="rvar")
                nc.vector.reciprocal(rvar[:], var[:])
                rln = st_pool.tile([P, 1], f32, tag="rln")
                nc.scalar.activation(rln[:], rvar[:], AF.Sqrt)
                osb = o_pool.tile([P, D], f32, tag="osb")
                for nh in range(2):
                    nc.vector.scalar_tensor_tensor(
                        osb[:, ts(nh, 512)], hc[nh][:], rln[:],
                        gamma_b[:, ts(nh, 512)], op0=OP.mult, op1=OP.mult)
                    nc.vector.tensor_add(osb[:, ts(nh, 512)],
                                         osb[:, ts(nh, 512)],
                                         beta_b[:, ts(nh, 512)])
                nc.gpsimd.dma_start(io["out_d"][ts(si, P), :], osb[:])


def build_program():
    nc = bacc.Bacc("TRN2", target_bir_lowering=False, debug=False,
                   num_devices=NCORES)
    io = {
        "x_d": nc.dram_tensor("x", [SH, D], f32, kind="ExternalInput").ap(),
        "mask_d": nc.dram_tensor("mask", [SH, S], u8, kind="ExternalInput").ap(),
        "wq_d": nc.dram_tensor("wq", [D, D], f32, kind="ExternalInput").ap(),
        "wk_d": nc.dram_tensor("wk", [D, D], f32, kind="ExternalInput").ap(),
        "wv_d": nc.dram_tensor("wv", [D, D], f32, kind="ExternalInput").ap(),
        "wo_d": nc.dram_tensor("wo", [D, D], f32, kind="ExternalInput").ap(),
        "bq_d": nc.dram_tensor("bq", [1, D], f32, kind="ExternalInput").ap(),
        "bk_d": nc.dram_tensor("bk", [1, D], f32, kind="ExternalInput").ap(),
        "bv_d": nc.dram_tensor("bv", [1, D], f32, kind="ExternalInput").ap(),
        "bo_d": nc.dram_tensor("bo", [1, D], f32, kind="ExternalInput").ap(),
        "gamma_d": nc.dram_tensor("gamma", [1, D], f32,
                                  kind="ExternalInput").ap(),
        "beta_d": nc.dram_tensor("beta", [1, D], f32, kind="ExternalInput").ap(),
        "att_d": nc.dram_tensor("attn_out", [SH, S], f32,
                                kind="ExternalOutput").ap(),
        "out_d": nc.dram_tensor("out", [SH, D], f32, kind="ExternalOutput").ap(),
        "ident_dram": nc.inline_tensor(np.eye(P, dtype=np.float32),
                                       name="ident_c"),
        "ones_dram": nc.inline_tensor(np.ones((1, SH), dtype=np.float32),
                                      name="ones_c"),
    }
    with tile.TileContext(nc) as tc:
        _emit(nc, tc, io)
    nc.compile()
    return nc


_CACHED = None


def _get_program():
    global _CACHED
    if _CACHED is None:
        _CACHED = build_program()
    return _CACHED


def _make_in_maps(embed_word_vec, attn_mask, wq, bq, wk, bk, wv, bv, wo, bo,
                  gamma, beta):
    x = np.ascontiguousarray(np.asarray(embed_word_vec, dtype=np.float32))
    mask = np.ascontiguousarray(np.asarray(attn_mask)).view(np.uint8)
    w_ = {n: np.ascontiguousarray(np.asarray(v, dtype=np.float32))
          for n, v in (("wq", wq), ("wk", wk), ("wv", wv), ("wo", wo))}
    b_ = {n: np.ascontiguousarray(
              np.asarray(v, dtype=np.float32)).reshape(1, D)
          for n, v in (("bq", bq), ("bk", bk), ("bv", bv), ("bo", bo),
                       ("gamma", gamma), ("beta", beta))}
    in_maps = []
    for c in range(NCORES):
        m = {"x": x[c * SH:(c + 1) * SH], "mask": mask[c * SH:(c + 1) * SH]}
        m.update(w_)
        m.update(b_)
        in_maps.append(m)
    return in_maps


def _assemble(res):
    attention = np.concatenate(
        [res.results[c]["attn_out"] for c in range(NCORES)], axis=0)
    output = np.concatenate(
        [res.results[c]["out"] for c in range(NCORES)], axis=0)
    return (output, attention)


def kernel(embed_word_vec, attn_mask, wq, bq, wk, bk, wv, bv, wo, bo, gamma,
           beta):
    nc = _get_program()
    in_maps = _make_in_maps(embed_word_vec, attn_mask, wq, bq, wk, bk, wv, bv,
                            wo, bo, gamma, beta)
    res = run_bass_kernel_spmd(nc, in_maps, list(range(NCORES)))
    return _assemble(res)


# revision 34
# speedup vs baseline: 1.0386x; 1.0028x over previous
"""Fused attention block (QKV -> masked softmax -> AV+residual -> out-proj -> LN)
for one trn2 chip (8 NeuronCores), sequence-parallel over the 4096 rows.

Self-contained: hardcodes shapes/sharding; builds+compiles the Bass program on
first call and caches it.
"""
import sys

sys.path.insert(0, "/opt/trn_rl_repo")

from contextlib import ExitStack  # noqa: E402

import numpy as np  # noqa: E402

import concourse.bacc as bacc  # noqa: E402
import concourse.bass as bass  # noqa: E402
import concourse.tile as tile  # noqa: E402
from concourse import mybir  # noqa: E402
from concourse.bass_utils import run_bass_kernel_spmd  # noqa: E402

f32 = mybir.dt.float32
f32r = mybir.dt.float32r
u8 = mybir.dt.uint8
bf = mybir.dt.bfloat16
AF = mybir.ActivationFunctionType
OP = mybir.AluOpType
AX = mybir.AxisListType
ts = bass.ts

NCORES = 8
S = 4096          # full sequence
D = 1024          # d_model
SH = S // NCORES  # 512 rows per core
P = 128
NSI = SH // P     # 4 si-tiles per core
NK = D // P       # 8 d_model tiles
NCH = S // 512    # 8 score chunks of 512 columns
NSJ = S // P      # 32 sj-tiles
SCALE = 1.0 / 8.0  # 1/sqrt(n_dim=64) per reference
LN_EPS = 1e-5
NEG = -1.0e30


def _emit(nc, tc, io):
    with ExitStack() as es:
        # ---------- long-lived pools (stack-ordered) ----------
        cpool = es.enter_context(tc.tile_pool(name="consts", bufs=1))
        xst_pool = es.enter_context(tc.tile_pool(name="xst", bufs=1))
        st_pool = es.enter_context(tc.tile_pool(name="stats", bufs=2))
        dram_pool = es.enter_context(
            tc.tile_pool(name="dramp", bufs=1, space="DRAM"))

        # internal DRAM for the K^T / V all-gathers (pool tiles => dep-tracked)
        # p-major layouts: one contiguous run per partition per chunk, so a
        # 2MB stream DMA is 128 descriptors instead of 1024.  K^T (f32r) and
        # V (bf16; its rounding error is diluted ~100x by the residual) share
        # ONE bounce buffer and ONE all-gather: the collective queue is
        # serial, so two gathers would chain their ~90us latencies.
        KCOLS = 2 * NK * SH          # K^T occupies 8192 bf16 slots (f32r x4096)
        VCOLS = 2 * NSI * 512        # V occupies 4096 bf16 slots
        kv_sh = dram_pool.tile([P, KCOLS + VCOLS], bf, tag="kv_sh")
        kv_full = dram_pool.tile([NCORES, P, KCOLS + VCOLS], bf, tag="kv_full",
                                 addr_space="Shared")

        # dummy collective: absorbs the one-time collective-mesh startup
        # latency concurrently with the compute prologue
        dumm_in = dram_pool.tile([1, 16], f32, tag="dumm_in")
        dumm_out = dram_pool.tile([NCORES, 16], f32, tag="dumm_out")
        with tc.high_priority():
            nc.sync.dma_start(dumm_in[:], io["ones_dram"].ap()[0:1, 0:16])
            nc.gpsimd.collective_compute(
                "AllGather", OP.bypass,
                replica_groups=[list(range(NCORES))],
                ins=[dumm_in.opt()], outs=[dumm_out.opt()],
            )

        # constants
        ident32 = cpool.tile([P, P], f32, tag="id32")
        identr = cpool.tile([P, P], f32r, tag="idr")
        nc.sync.dma_start(ident32[:], io["ident_dram"].ap())
        nc.sync.dma_start(identr[:], io["ident_dram"].ap().bitcast(f32r))
        onesr = cpool.tile([1, SH], f32r, tag="onesr")
        nc.sync.dma_start(onesr[:], io["ones_dram"].ap().bitcast(f32r))
        ones_l = onesr[0:1, 0:P]     # K=1 stationary operand for rank-1 bias
        bvec = {}
        for n in ("bq", "bk", "bv", "bo", "gamma", "beta"):
            bvec[n] = cpool.tile([1, D], f32r, tag=n, name="bv_" + n)
            nc.sync.dma_start(bvec[n][:], io[n + "_d"].bitcast(f32r))

        # broadcast gamma/beta to [P, D] via rank-1 matmuls
        gamma_b = cpool.tile([P, D], bf, tag="gammab")
        beta_b = cpool.tile([P, D], bf, tag="betab")
        with tc.tile_pool(name="psgb", bufs=2, space="PSUM") as psgb:
            for half in range(2):
                pg = psgb.tile([P, 512], f32, tag="pg")
                nc.tensor.matmul(pg[:], ones_l, bvec["gamma"][0:1, ts(half, 512)],
                                 start=True, stop=True)
                nc.vector.tensor_copy(gamma_b[:, ts(half, 512)], pg[:])
                pb = psgb.tile([P, 512], f32, tag="pb")
                nc.tensor.matmul(pb[:], ones_l, bvec["beta"][0:1, ts(half, 512)],
                                 start=True, stop=True)
                nc.vector.tensor_copy(beta_b[:, ts(half, 512)], pb[:])

        # persistent SBUF tensors
        xstr = [xst_pool.tile([P, SH], f32r, tag=f"xstr_{k}", name=f"xstr_{k}") for k in range(NK)]
        sums = [st_pool.tile([P, NCH], f32, tag=f"sums{si}", name=f"sums{si}") for si in range(NSI)]
        r_bcast = st_pool.tile([P, SH], f32, tag="r_bcast", name="r_bcast")
        rrec = [st_pool.tile([P, 1], f32, tag=f"rrec{si}", name=f"rrec{si}") for si in range(NSI)]

        # ---------- phase T: load X shard, transpose ----------
        with (
            nc.named_scope("phaseT"),
            tc.tile_pool(name="xs", bufs=1) as xs_pool,
            tc.tile_pool(name="psx", bufs=2, space="PSUM") as psx_pool,
        ):
            xs = [xs_pool.tile([P, D], f32, tag=f"xs{si}", name=f"xs{si}") for si in range(NSI)]
            for si in range(NSI):
                nc.sync.dma_start(xs[si][:], io["x_d"][ts(si, P), :])
            for k in range(NK):
                psx = psx_pool.tile([P, SH], f32, tag="psx")
                for si in range(NSI):
                    nc.tensor.transpose(psx[:, ts(si, P)], xs[si][:, ts(k, P)],
                                        ident32[:])
                nc.vector.tensor_copy(xstr[k][:], psx[:])

        # ---------- phase QKV ----------
        qt_pool = es.enter_context(tc.tile_pool(name="qtpool", bufs=1))
        qt_t = [qt_pool.tile([P, SH], f32r, tag=f"qt{m}", name=f"qtt{m}")
                for m in range(NK)]
        with (
            nc.named_scope("qkv"),
            tc.tile_pool(name="wsl", bufs=16) as w_pool,
            tc.tile_pool(name="ktloc", bufs=1) as ktl_pool,
            tc.tile_pool(name="vsb", bufs=1) as v_sb_pool,
            tc.tile_pool(name="psqk", bufs=3, space="PSUM") as psqk_pool,
        ):
            # K^T then V, then ONE combined all-gather overlapping Q compute.
            # Weights DMA'd as full [128, 1024] rows (4KB/partition) --
            # small column slices would run the queues at ~40% efficiency.
            ktbig = ktl_pool.tile([P, NK * SH], f32r, tag="ktbig",
                                  name="ktbig")
            ktloc = [ktbig[:, ts(m, SH)] for m in range(NK)]
            wkt = []
            for k in range(NK):
                w1 = w_pool.tile([P, D], f32r, tag="wfull")
                nc.sync.dma_start(w1[:], io["wk_d"][ts(k, P), :].bitcast(f32r))
                wkt.append(w1)
            for m in range(NK):
                ps = psqk_pool.tile([P, SH], f32, tag="psqk")
                for k in range(NK):
                    nc.tensor.matmul(ps[:], wkt[k][:, ts(m, P)], xstr[k][:],
                                     start=(k == 0), stop=False)
                nc.tensor.matmul(ps[:], bvec["bk"][0:1, ts(m, P)], onesr[:],
                                 start=False, stop=True)
                with tc.high_priority():
                    nc.vector.tensor_copy(ktloc[m], ps[:])
            with tc.high_priority():
                nc.scalar.dma_start(kv_sh[:, 0:KCOLS].bitcast(f32r), ktbig[:])
            # V in natural layout: out[si-tile, n] = X @ Wv  (bf16 bounce)
            wvt = []
            for k in range(NK):
                w1 = w_pool.tile([P, D], f32r, tag="wfull")
                nc.sync.dma_start(w1[:], io["wv_d"][ts(k, P), :].bitcast(f32r))
                wvt.append(w1)
            vbig = v_sb_pool.tile([P, NSI * 2 * 512], bf, tag="vbig",
                                  name="vbig")
            for si in range(NSI):
                for nh in range(2):
                    ps = psqk_pool.tile([P, SH], f32, tag="psqk")
                    for k in range(NK):
                        nc.tensor.matmul(ps[:], xstr[k][:, ts(si, P)],
                                         wvt[k][:, ts(nh, 512)],
                                         start=(k == 0), stop=False)
                    nc.tensor.matmul(ps[:], ones_l, bvec["bv"][0:1, ts(nh, 512)],
                                     start=False, stop=True)
                    with tc.high_priority():
                        nc.vector.tensor_copy(vbig[:, ts(nh * NSI + si, 512)],
                                              ps[:])
            with tc.high_priority():
                nc.sync.dma_start(kv_sh[:, KCOLS:], vbig[:])
                nc.gpsimd.collective_compute(
                    "AllGather", OP.bypass,
                    replica_groups=[list(range(NCORES))],
                    ins=[kv_sh.opt()], outs=[kv_full.opt()],
                )
            # Q^T last: overlaps the gather
            wqt = []
            for k in range(NK):
                w1 = w_pool.tile([P, D], f32r, tag="wfull")
                nc.sync.dma_start(w1[:], io["wq_d"][ts(k, P), :].bitcast(f32r))
                wqt.append(w1)
            for m in range(NK):
                ps = psqk_pool.tile([P, SH], f32, tag="psqk")
                for k in range(NK):
                    nc.tensor.matmul(ps[:], wqt[k][:, ts(m, P)], xstr[k][:],
                                     start=(k == 0), stop=False)
                nc.tensor.matmul(ps[:], bvec["bq"][0:1, ts(m, P)], onesr[:],
                                 start=False, stop=True)
                nc.vector.tensor_copy(qt_t[m][:], ps[:])

        # ---------- phase scores + softmax ----------
        a_pool = es.enter_context(tc.tile_pool(name="apool", bufs=1))
        a_t = [a_pool.tile([P, S], f32r, tag=f"a{si}", name=f"a{si}")
               for si in range(NSI)]
        masks = []
        with (
            nc.named_scope("scores"),
            tc.tile_pool(name="ktst", bufs=3) as kt_pool,
            tc.tile_pool(name="mrot", bufs=1) as m_pool,
            tc.tile_pool(name="pss", bufs=1, space="PSUM") as pss_pool,
        ):
            negc_b = m_pool.tile([P, 512], f32, tag="negc")
            nc.vector.memset(negc_b[:], NEG / SCALE)
            for c in range(NCH):
                # one batched DMA for the whole K^T chunk (8 m-tiles)
                kt = kt_pool.tile([P, NK * SH], f32r, tag="kt")
                eng = nc.scalar if c % 2 == 0 else nc.sync
                eng.dma_start(kt[:], kv_full[c][:, 0:KCOLS].bitcast(f32r))
                if c == 0:
                    for si in range(NSI):
                        m1 = m_pool.tile([P, S], u8, tag=f"mask{si}",
                                         name=f"mask{si}")
                        nc.sync.dma_start(m1[:], io["mask_d"][ts(si, P), :])
                        masks.append(m1)
                for si in range(NSI):
                    ps = pss_pool.tile([P, 512], f32, tag=f"pss{si}")
                    for m in range(NK):
                        nc.tensor.matmul(ps[:], qt_t[m][:, ts(si, P)],
                                         kt[:, ts(m, SH)],
                                         start=(m == 0), stop=(m == NK - 1))
                    # masked scores -> -1e30; then exp(SCALE * x)
                    nc.vector.copy_predicated(ps[:], masks[si][:, ts(c, 512)],
                                              negc_b[:])
                    nc.scalar.activation(
                        a_t[si][:, ts(c, 512)], ps[:], AF.Exp,
                        scale=SCALE, accum_out=sums[si][:, c:c + 1])
            # row-sum reciprocals; build r broadcast [P, 512] along the
            # si free dim (rank-1) so C^T can be scaled without normalizing
            # A first -- the AV transposes then need only the raw exp values.
            for si in range(NSI):
                lsum = st_pool.tile([P, 1], f32, tag="lsum")
                nc.vector.tensor_reduce(lsum[:], sums[si][:], AX.X, OP.add)
                nc.vector.reciprocal(rrec[si][:], lsum[:])
            r4 = st_pool.tile([P, 4], f32, tag="r4")
            for si in range(NSI):
                nc.vector.tensor_copy(r4[:, si:si + 1], rrec[si][:])
            ps_r = pss_pool.tile([P, 512], f32, tag="pss0")
            nc.tensor.transpose(ps_r[0:4, 0:P], r4[:], ident32[:])
            r4t = st_pool.tile([4, P], f32, tag="r4t")
            nc.vector.tensor_copy(r4t[:], ps_r[0:4, 0:P])
            r_row = st_pool.tile([1, SH], f32r, tag="r_row")
            for si in range(NSI):
                nc.sync.dma_start(r_row[0:1, ts(si, P)].bitcast(f32),
                                  r4t[si:si + 1, :])
            ps_rb = pss_pool.tile([P, 512], f32, tag="pss1")
            nc.tensor.matmul(ps_rb[:], ones_l, r_row[:], start=True, stop=True)
            nc.vector.tensor_copy(r_bcast[:], ps_rb[:])

        # ---------- phase AV: C^T += V^T slice @ A^T, 2 column halves ----------
        ct_pool = es.enter_context(tc.tile_pool(name="ctpool", bufs=1))
        ct_t = [ct_pool.tile([P, SH], f32r, tag=f"ct{k}", name=f"ctt{k}")
                for k in range(NK)]
        wo_pool = es.enter_context(tc.tile_pool(name="wo", bufs=1))
        wo_sl = []
        for nh in range(2):
            for k in range(NK):
                w1 = wo_pool.tile([P, 512], f32r, tag=f"wo{nh}_{k}",
                                  name=f"wo{nh}_{k}")
                nc.sync.dma_start(
                    w1[:], io["wo_d"][ts(k, P), ts(nh, 512)].bitcast(f32r))
                wo_sl.append(w1)
        with (
            nc.named_scope("av"),
            tc.tile_pool(name="etrot", bufs=2) as et_pool,
            tc.tile_pool(name="vrot", bufs=3) as vr_pool,
            tc.tile_pool(name="psct", bufs=1, space="PSUM") as psct_pool,
            tc.tile_pool(name="pstr", bufs=2, space="PSUM") as pstr_pool,
        ):
            for h in range(2):
                psct = [psct_pool.tile([P, SH], f32, tag=f"psct{nt}", name=f"psct{nt}")
                        for nt in range(4)]
                for sj in range(NSJ):
                    if sj % 4 == 0:
                        # batched V DMA: this core-chunk's half (4 sj-tiles)
                        vt = vr_pool.tile([P, 4 * 512], bf, tag="vt")
                        veng = nc.scalar if (sj // 4) % 2 == 0 else nc.sync
                        veng.dma_start(
                            vt[:],
                            kv_full[sj // 4][:, KCOLS + h * 2048:
                                             KCOLS + (h + 1) * 2048])
                    vs = vt[:, ts(sj % 4, 512)]
                    pst = pstr_pool.tile([P, SH], f32r, tag="pstr")
                    for si in range(NSI):
                        nc.tensor.transpose(pst[:, ts(si, P)],
                                            a_t[si][:, ts(sj, P)], identr[:])
                    ett = et_pool.tile([P, SH], bf, tag="ett")
                    nc.vector.tensor_copy(ett[:], pst[:].bitcast(f32))
                    for nt in range(4):
                        nc.tensor.matmul(psct[nt][:],
                                         vs[:, ts(nt, P)], ett[:],
                                         start=(sj == 0), stop=(sj == NSJ - 1))
                for nt in range(4):
                    k = h * 4 + nt
                    # C^T = r * (E@V)^T + X^T  (softmax scale + residual)
                    nc.vector.tensor_mul(psct[nt][:], psct[nt][:], r_bcast[:])
                    nc.vector.tensor_add(ct_t[k][:], psct[nt][:], xstr[k][:])

        # normalize A for the attention output (runs during/after AV; the
        # WAR on the AV transposes orders it automatically)
        for si in range(NSI):
            nc.vector.tensor_scalar_mul(a_t[si][:], a_t[si][:], rrec[si][:])
            nc.gpsimd.dma_start(io["att_d"][ts(si, P), :],
                                a_t[si][:].bitcast(f32))

        # ---------- phase out-proj + LayerNorm ----------
        with (
            nc.named_scope("hln"),
            tc.tile_pool(name="hc", bufs=2) as hc_pool,
            tc.tile_pool(name="osb", bufs=2) as o_pool,
            tc.tile_pool(name="scr", bufs=2) as scr_pool,
            tc.tile_pool(name="psh", bufs=1, space="PSUM") as psh_pool,
        ):
            psh = [[psh_pool.tile([P, 512], f32, tag=f"psh{si}_{nh}", name=f"psh{si}_{nh}")
                    for nh in range(2)] for si in range(NSI)]
            inv_d = 1.0 / D
            for shalf in range(2):
                sis = (shalf * 2, shalf * 2 + 1)
                for nh in range(2):
                    for k in range(NK):
                        for si in sis:
                            nc.tensor.matmul(psh[si][nh][:], ct_t[k][:, ts(si, P)],
                                             wo_sl[nh * NK + k][:],
                                             start=(k == 0), stop=False)
                    for si in sis:
                        nc.tensor.matmul(psh[si][nh][:], ones_l,
                                         bvec["bo"][0:1, ts(nh, 512)],
                                         start=False, stop=True)
            for si in range(NSI):
                st2 = st_pool.tile([P, 2], f32, tag="st2")
                for nh in range(2):
                    nc.vector.tensor_reduce(st2[:, nh:nh + 1], psh[si][nh][:],
                                            AX.X, OP.add)
                mu = st_pool.tile([P, 1], f32, tag="mu")
                nc.vector.tensor_add(mu[:], st2[:, 0:1], st2[:, 1:2])
                nc.vector.tensor_scalar_mul(mu[:], mu[:], inv_d)
                hc = [hc_pool.tile([P, 512], f32, tag=f"hc{nh}", name=f"hc{nh}")
                      for nh in range(2)]
                ss2 = st_pool.tile([P, 2], f32, tag="ss2")
                for nh in range(2):
                    nc.vector.tensor_scalar(hc[nh][:], psh[si][nh][:], mu[:],
                                            None, op0=OP.subtract)
                    scr = scr_pool.tile([P, 512], f32, tag="scr")
                    nc.scalar.activation(scr[:], hc[nh][:], AF.Square,
                                         accum_out=ss2[:, nh:nh + 1])
                var = st_pool.tile([P, 1], f32, tag="var")
                nc.vector.tensor_add(var[:], ss2[:, 0:1], ss2[:, 1:2])
                nc.vector.tensor_scalar(var[:], var[:], inv_d, LN_EPS,
                                        op0=OP.mult, op1=OP.add)
                rvar = st_pool.tile([P, 1], f32, tag="rvar")
                nc.vector.reciprocal(rvar[:], var[:])
                rln = st_pool.tile([P, 1], f32, tag="rln")
                nc.scalar.activation(rln[:], rvar[:], AF.Sqrt)
                osb = o_pool.tile([P, D], f32, tag="osb")
                for nh in range(2):
                    nc.vector.scalar_tensor_tensor(
                        osb[:, ts(nh, 512)], hc[nh][:], rln[:],
                        gamma_b[:, ts(nh, 512)], op0=OP.mult, op1=OP.mult)
                    nc.vector.tensor_add(osb[:, ts(nh, 512)],
                                         osb[:, ts(nh, 512)],
                                         beta_b[:, ts(nh, 512)])
                nc.gpsimd.dma_start(io["out_d"][ts(si, P), :], osb[:])


def build_program():
    nc = bacc.Bacc("TRN2", target_bir_lowering=False, debug=False,
                   num_devices=NCORES)
    io = {
        "x_d": nc.dram_tensor("x", [SH, D], f32, kind="ExternalInput").ap(),
        "mask_d": nc.dram_tensor("mask", [SH, S], u8, kind="ExternalInput").ap(),
        "wq_d": nc.dram_tensor("wq", [D, D], f32, kind="ExternalInput").ap(),
        "wk_d": nc.dram_tensor("wk", [D, D], f32, kind="ExternalInput").ap(),
        "wv_d": nc.dram_tensor("wv", [D, D], f32, kind="ExternalInput").ap(),
        "wo_d": nc.dram_tensor("wo", [D, D], f32, kind="ExternalInput").ap(),
        "bq_d": nc.dram_tensor("bq", [1, D], f32, kind="ExternalInput").ap(),
        "bk_d": nc.dram_tensor("bk", [1, D], f32, kind="ExternalInput").ap(),
        "bv_d": nc.dram_tensor("bv", [1, D], f32, kind="ExternalInput").ap(),
        "bo_d": nc.dram_tensor("bo", [1, D], f32, kind="ExternalInput").ap(),
        "gamma_d": nc.dram_tensor("gamma", [1, D], f32,
                                  kind="ExternalInput").ap(),
        "beta_d": nc.dram_tensor("beta", [1, D], f32, kind="ExternalInput").ap(),
        "att_d": nc.dram_tensor("attn_out", [SH, S], f32,
                                kind="ExternalOutput").ap(),
        "out_d": nc.dram_tensor("out", [SH, D], f32, kind="ExternalOutput").ap(),
        "ident_dram": nc.inline_tensor(np.eye(P, dtype=np.float32),
                                       name="ident_c"),
        "ones_dram": nc.inline_tensor(np.ones((1, SH), dtype=np.float32),
                                      name="ones_c"),
    }
    with tile.TileContext(nc) as tc:
        _emit(nc, tc, io)
    nc.compile()
    return nc


_CACHED = None


def _get_program():
    global _CACHED
    if _CACHED is None:
        _CACHED = build_program()
    return _CACHED


def _make_in_maps(embed_word_vec, attn_mask, wq, bq, wk, bk, wv, bv, wo, bo,
                  gamma, beta):
    x = np.ascontiguousarray(np.asarray(embed_word_vec, dtype=np.float32))
    mask = np.ascontiguousarray(np.asarray(attn_mask)).view(np.uint8)
    w_ = {n: np.ascontiguousarray(np.asarray(v, dtype=np.float32))
          for n, v in (("wq", wq), ("wk", wk), ("wv", wv), ("wo", wo))}
    b_ = {n: np.ascontiguousarray(
              np.asarray(v, dtype=np.float32)).reshape(1, D)
          for n, v in (("bq", bq), ("bk", bk), ("bv", bv), ("bo", bo),
                       ("gamma", gamma), ("beta", beta))}
    in_maps = []
    for c in range(NCORES):
        m = {"x": x[c * SH:(c + 1) * SH], "mask": mask[c * SH:(c + 1) * SH]}
        m.update(w_)
        m.update(b_)
        in_maps.append(m)
    return in_maps


def _assemble(res):
    attention = np.concatenate(
        [res.results[c]["attn_out"] for c in range(NCORES)], axis=0)
    output = np.concatenate(
        [res.results[c]["out"] for c in range(NCORES)], axis=0)
    return (output, attention)


def kernel(embed_word_vec, attn_mask, wq, bq, wk, bk, wv, bv, wo, bo, gamma,
           beta):
    nc = _get_program()
    in_maps = _make_in_maps(embed_word_vec, attn_mask, wq, bq, wk, bk, wv, bv,
                            wo, bo, gamma, beta)
    res = run_bass_kernel_spmd(nc, in_maps, list(range(NCORES)))
    return _assemble(res)
